# revision 3
# baseline (speedup 1.0000x reference)
"""Bass/Trainium2 kernel for nn_BayesMultiheadAttention (B=4,T=2048,D=1024,H=8).

Sharding: tensor-parallel over heads. Core c computes head c (QKV proj +
causal attention) for all 4 batches; a per-batch AllToAll redistributes
per-head outputs into per-token-slice outputs (pipelined against the next
batch's compute); each core then does the multiplicative reduce over heads
and its slice of out_proj.

x and the QKV weights are converted to bf16 on the host and DMA'd straight
into SBUF (no on-chip rounding passes); all projection/attention matmuls
run in bf16 (1 cycle/row at any free size), accumulating fp32 in PSUM.
Scores, softmax normalization, the AllToAll payload, the head product and
out_proj stay fp32/f32r. Softmax denominators are accumulated in PSUM by
per-tile ones-matmuls. Dropout masks and the 1/sqrt(HD) scale are folded
into per-(core,batch) weight copies on the host. Softmax skips
max-subtraction (scores are O(5), exp cannot overflow).
"""
import numpy as np

import concourse.bacc as bacc
import concourse.mybir as mybir
import concourse.tile as tile
from concourse.bass_utils import run_bass_kernel_spmd

B, T, D, H = 4, 2048, 1024, 8
HD = 128          # head dim
P = 128           # partitions
NC = 8            # cores
TQ = 512          # qt chunk width
NKD = D // P      # 8 contraction tiles
NTT = T // P      # 16 token tiles per batch
NQC = T // TQ     # 4 qt chunks per batch
TS = T // NC      # 256: per-core token slice of one batch
TOK_SLICE = B * TS  # 1024 tokens per core in the tail

dt = mybir.dt
F32 = dt.float32
F32R = dt.float32r
BF16 = dt.bfloat16

_PROGRAM = {}


def build_program(nreps=1):
    global _PROGRAM
    if nreps in _PROGRAM:
        return _PROGRAM[nreps]
    nc = bacc.Bacc("TRN2", target_bir_lowering=False, debug=False,
                   num_devices=NC)

    xT_d = nc.dram_tensor("xT", [B, D, T], BF16, kind="ExternalInput")
    wq_d = nc.dram_tensor("wq", [B, NKD, P, HD], BF16, kind="ExternalInput")
    wk_d = nc.dram_tensor("wk", [B, NKD, P, HD], BF16, kind="ExternalInput")
    wv_d = nc.dram_tensor("wv", [B, NKD, P, HD], BF16, kind="ExternalInput")
    wo_d = nc.dram_tensor("wo", [HD, D], F32, kind="ExternalInput")
    cm_d = nc.dram_tensor("cm", [4, P, TQ], BF16, kind="ExternalInput")
    eye_d = nc.dram_tensor("eye", [P, P], BF16, kind="ExternalInput")
    y_d = nc.dram_tensor("y", [TOK_SLICE, D], F32, kind="ExternalOutput")

    rg = [list(range(NC))]
    Exp = mybir.ActivationFunctionType.Exp

    from contextlib import ExitStack
    with tile.TileContext(nc) as tc, ExitStack() as ctx:
        ec = ctx.enter_context
        constp = ec(tc.tile_pool(name="const", bufs=1))
        xrp = ec(tc.tile_pool(name="xr", bufs=1))
        wrp = ec(tc.tile_pool(name="wr", bufs=1))
        qkvp = ec(tc.tile_pool(name="qkv", bufs=1))
        eop = ec(tc.tile_pool(name="eo", bufs=6))
        scp = ec(tc.tile_pool(name="sc", bufs=3))
        outbp = ec(tc.tile_pool(name="outb", bufs=1))
        tailp = ec(tc.tile_pool(name="tail", bufs=2))
        hpp = ec(tc.tile_pool(name="hp", bufs=2))
        ysbp = ec(tc.tile_pool(name="ysb", bufs=2))
        psA = ec(tc.tile_pool(name="psA", bufs=2, space="PSUM"))
        psS = ec(tc.tile_pool(name="psS", bufs=2, space="PSUM"))
        psO = ec(tc.tile_pool(name="psO", bufs=2, space="PSUM"))
        psD = ec(tc.tile_pool(name="psD", bufs=2, space="PSUM"))
        dram = ec(tc.tile_pool(name="dram", bufs=1, space="DRAM"))
        a2a_in = [dram.tile([NC, P, TS], F32, name=f"a2a_in{b}",
                            tag=f"a2a_in{b}") for b in range(B)]
        a2a_out = [dram.tile([NC, P, TS], F32, name=f"a2a_out{b}",
                             tag=f"a2a_out{b}") for b in range(B)]

        ones_b = constp.tile([P, P], BF16, name="ones_b", tag="ones_b")
        nc.vector.memset(ones_b[:], 1.0)

        eye_b = constp.tile([P, P], BF16, name="eye_b", tag="eye_b")
        nc.sync.dma_start(eye_b[:], eye_d.ap())

        cm_sb = constp.tile([P, 4 * TQ], BF16, name="cm_sb", tag="cm_sb")
        nc.sync.dma_start(cm_sb[:], cm_d.ap().rearrange("j p q -> p j q"))

        wo_st = constp.tile([P, D], F32, name="wo_st", tag="wo_st")
        nc.sync.dma_start(wo_st[:], wo_d.ap())
        wor = constp.tile([P, D], F32R, name="wor", tag="wor")
        nc.vector.tensor_copy(wor[:], wo_st[:])

        prodr = tailp.tile([P, TOK_SLICE], F32R, name="prodr",
                           tag="prodr", bufs=1)

        tail_pr = {}

        def emit_tail_head(b):
            """Start consuming A2A(b): head product chain on Pool."""
            hp = hpp.tile([P, NC * TS], F32, name="hp", tag="hp")
            nc.gpsimd.dma_start(
                hp[:], a2a_out[b].rearrange("r p t -> p r t"))
            pr = tailp.tile([P, TS], F32, name="pr", tag="pr")
            nc.gpsimd.tensor_mul(pr[:], hp[:, 0:TS], hp[:, TS:2 * TS])
            for r in range(2, NC - 1):
                nc.gpsimd.tensor_mul(
                    pr[:], pr[:], hp[:, r * TS:(r + 1) * TS])
            tail_pr[b] = (pr, hp)

        def emit_tail_tail(b):
            """Finish A2A(b): final product multiply + out_proj slice."""
            pr, hp = tail_pr.pop(b)
            nc.vector.tensor_mul(
                prodr[:, b * TS:(b + 1) * TS], pr[:],
                hp[:, (NC - 1) * TS:NC * TS])
            for ttl in range(TS // P):
                tt = b * (TS // P) + ttl
                ysb = ysbp.tile([P, D], F32, name="ysb", tag="ysb")
                for nn in range(D // TQ):
                    accy = psA.tile([P, TQ], F32, name="accy",
                                    tag="mmacc")
                    nc.tensor.matmul(
                        accy[:],
                        prodr[:, tt * P:(tt + 1) * P],
                        wor[:, nn * TQ:(nn + 1) * TQ],
                        start=True, stop=True)
                    nc.vector.tensor_copy(
                        ysb[:, nn * TQ:(nn + 1) * TQ], accy[:])
                nc.sync.dma_start(y_d.ap()[tt * P:(tt + 1) * P, :],
                                  ysb[:])

        staged = {}
        pending = {}

        def make_load_steps(b):
            """Closures that DMA batch b's x and weights into SBUF (bf16)."""
            st = {"wr": {}}
            staged[b] = st

            def mk_x(kd):
                def f():
                    if "xr" not in st:
                        st["xr"] = xrp.tile([P, NKD * T], BF16,
                                            name="xr", tag="xr")
                    eng = nc.sync if kd % 2 == 0 else nc.gpsimd
                    eng.dma_start(st["xr"][:, kd * T:(kd + 1) * T],
                                  xT_d.ap()[b, kd * P:(kd + 1) * P, :])
                return f

            def mk_w(nm, wd):
                def f():
                    wt = wrp.tile([P, NKD * HD], BF16, name=f"wr_{nm}",
                                  tag=f"wr_{nm}")
                    half = NKD // 2 * HD
                    rr = wd.ap()[b].rearrange("kd p m -> p kd m")
                    nc.sync.dma_start(wt[:, 0:half], rr[:, 0:NKD // 2])
                    nc.gpsimd.dma_start(wt[:, half:], rr[:, NKD // 2:])
                    st["wr"][nm] = wt
                return f

            wsteps = [mk_w(nm, wd)
                      for nm, wd in (("v", wv_d), ("q", wq_d),
                                     ("k", wk_d))]
            xsteps = [mk_x(kd) for kd in range(NKD)]
            steps = [wsteps[0], xsteps[0], xsteps[1], wsteps[1],
                     xsteps[2], xsteps[3], wsteps[2]] + xsteps[4:]
            return steps

        def drain_pending(b, n=None):
            steps = pending.get(b, [])
            k = len(steps) if n is None else min(n, len(steps))
            for f in steps[:k]:
                f()
            pending[b] = steps[k:]

        for rep in range(nreps):
            first = rep == 0
            for b in range(B):
                if b == 0:
                    if first:
                        pending[0] = make_load_steps(0)
                    drain_pending(0)
                else:
                    drain_pending(b)
                st = staged[b]
                xr = st["xr"]

                # ---- projections: v first, then q, k
                qkt = {}
                v_sb = None
                for nm in ("v", "q", "k"):
                    wt = st["wr"][nm]

                    dest = qkvp.tile([P, T], BF16, name=f"{nm}T",
                                     tag=f"{nm}T")
                    if b == 0 and first and nm == "v":
                        # kd-outer: start PE as soon as the first x tile
                        # lands; 4 chunk accumulators across spare banks
                        accs4 = [
                            (psS if i < 2 else psO).tile(
                                [P, TQ], F32, name=f"pacc{i}",
                                tag="accs" if i < 2 else "acco")
                            for i in range(NQC)]
                        for kd in range(NKD):
                            for qc in range(NQC):
                                nc.tensor.matmul(
                                    accs4[qc][:],
                                    wt[:, kd * HD:(kd + 1) * HD],
                                    xr[:, kd * T + qc * TQ: kd * T + (qc + 1) * TQ],
                                    start=(kd == 0), stop=(kd == NKD - 1))
                        for qc in range(NQC):
                            nc.vector.tensor_copy(
                                dest[:, qc * TQ:(qc + 1) * TQ], accs4[qc][:])
                    else:
                        for qc in range(NQC):
                            acc = psA.tile([P, TQ], F32, name="acc",
                                           tag="mmacc")
                            for kd in range(NKD):
                                nc.tensor.matmul(
                                    acc[:],
                                    wt[:, kd * HD:(kd + 1) * HD],
                                    xr[:, kd * T + qc * TQ: kd * T + (qc + 1) * TQ],
                                    start=(kd == 0), stop=(kd == NKD - 1))
                            nc.vector.tensor_copy(
                                dest[:, qc * TQ:(qc + 1) * TQ], acc[:])
                    qkt[nm] = dest

                    if nm == "v":
                        # flip V to (tok parts, hd free) via PE transposes
                        v_sb = qkvp.tile([P, NTT * HD], BF16, name="vS",
                                         tag="vS")
                        for tt in range(NTT):
                            vtp = psA.tile([P, P], BF16, name="vtp",
                                           tag="mmacc")
                            nc.tensor.transpose(
                                vtp[:], dest[:, tt * P:(tt + 1) * P],
                                eye_b[:])
                            nc.vector.tensor_copy(
                                v_sb[:, tt * HD:(tt + 1) * HD], vtp[:])

                # ---- causal attention, scoresT layout ----
                out_b = outbp.tile([P, T], F32, name="out_b", tag="out_b")
                for qc in range(NQC):
                    nkt = 4 * (qc + 1)
                    acco = psO.tile([P, TQ], F32, name="acco", tag="acco")
                    denb = psD.tile([P, TQ], F32, name="denb", tag="denb")
                    for kt in range(nkt):
                        accs = psS.tile([P, TQ], F32, name="accs",
                                        tag="accs")
                        nc.tensor.matmul(
                            accs[:],
                            qkt["k"][:, kt * P:(kt + 1) * P],
                            qkt["q"][:, qc * TQ:(qc + 1) * TQ],
                            start=True, stop=True)
                        e = eop.tile([P, TQ], BF16, name="e", tag="e")
                        nc.scalar.activation(e[:], accs[:], Exp)
                        j = kt - 4 * qc
                        if j >= 0:  # diagonal-crossing tile: zero invalid
                            nc.vector.tensor_mul(
                                e[:], e[:], cm_sb[:, j * TQ:(j + 1) * TQ])
                        nc.tensor.matmul(
                            acco[:],
                            v_sb[:, kt * HD:(kt + 1) * HD],
                            e[:],
                            start=(kt == 0), stop=(kt == nkt - 1))
                        nc.tensor.matmul(
                            denb[:], ones_b[:], e[:],
                            start=(kt == 0), stop=(kt == nkt - 1))
                    recb = scp.tile([P, TQ], F32, name="recb", tag="recb")
                    nc.vector.reciprocal_approx_fast(recb[:], denb[:])
                    nc.vector.tensor_mul(
                        out_b[:, qc * TQ:(qc + 1) * TQ], acco[:], recb[:])

                    if qc == 1 and not (b == 0 and first):
                        pb = b - 1 if b > 0 else B - 1
                        emit_tail_tail(pb)
                    if b + 1 < B:
                        if qc == 0:
                            pending[b + 1] = make_load_steps(b + 1)
                        drain_pending(b + 1, 4)
                    elif rep + 1 < nreps:
                        if qc == 0:
                            pending[0] = make_load_steps(0)
                        drain_pending(0, 4)

                # ---- ship normalized head-output
                for j in range(NC):
                    nc.sync.dma_start(a2a_in[b][j],
                                      out_b[:, j * TS:(j + 1) * TS])
                nc.gpsimd.collective_compute(
                    "AllToAll", mybir.AluOpType.bypass,
                    replica_groups=rg,
                    ins=[a2a_in[b].opt()], outs=[a2a_out[b].opt()])
                emit_tail_head(b)

        emit_tail_tail(B - 1)

    nc.compile()
    _PROGRAM[nreps] = nc
    return nc


def make_in_maps(x, Wq, Wk, Wv, Wout, q_mask, k_mask, v_mask):
    import ml_dtypes
    bf16 = ml_dtypes.bfloat16
    x = np.asarray(x, np.float32)
    xT = np.ascontiguousarray(x.transpose(0, 2, 1)).astype(bf16)  # (B, D, T)
    wo = np.ascontiguousarray(np.asarray(Wout, np.float32).T)  # (HD, D)

    cm = np.zeros((4, P, TQ), np.float32)
    for j in range(4):
        for i in range(P):
            cm[j, i, j * P + i:] = 1.0
    cm = cm.astype(bf16)
    eye = np.eye(P, dtype=np.float32).astype(bf16)

    s = np.float32(1.0 / np.sqrt(HD))
    q_mask = np.asarray(q_mask, np.float32)
    k_mask = np.asarray(k_mask, np.float32)
    v_mask = np.asarray(v_mask, np.float32)
    Wq = np.asarray(Wq, np.float32)
    Wk = np.asarray(Wk, np.float32)
    Wv = np.asarray(Wv, np.float32)

    in_maps = []
    for c in range(NC):
        def pack(W, m, scale):
            out = np.empty((B, NKD, P, HD), np.float32)
            Wh = W[c * HD:(c + 1) * HD, :]                  # (HD, D)
            for b in range(B):
                Wp = (Wh * (m[b, c, 0, :, None] * scale)).T  # (D, HD)
                out[b] = Wp.reshape(NKD, P, HD)
            return out.astype(bf16)
        in_maps.append({
            "xT": xT,
            "wq": pack(Wq, q_mask, s),
            "wk": pack(Wk, k_mask, np.float32(1.0)),
            "wv": pack(Wv, v_mask, np.float32(1.0)),
            "wo": wo,
            "cm": cm,
            "eye": eye,
        })
    return in_maps


def kernel(x, Wq, Wk, Wv, Wout, q_mask, k_mask, v_mask, mask=None):
    nc = build_program()
    in_maps = make_in_maps(x, Wq, Wk, Wv, Wout, q_mask, k_mask, v_mask)
    res = run_bass_kernel_spmd(nc, in_maps, core_ids=list(range(NC))).results
    # core c's y rows are ordered (b, local-token); its tokens are
    # [c*TS, (c+1)*TS) of every batch
    out = np.empty((B, T, D), np.float32)
    for c in range(NC):
        yc = res[c]["y"].reshape(B, TS, D)
        out[:, c * TS:(c + 1) * TS, :] = yc
    return out
